# revision 61
# baseline (speedup 1.0000x reference)
"""KIVI attention wrapper — Trainium2 Bass kernel, 8-way head-sharded.

Sharding: 16 heads / 8 cores = 2 heads per core (tensor parallel) through
attention; c_proj is token-sharded (each core computes the full 1024 output
features for its 512-token slab) fed by an AllToAll of the per-head attention
outputs — 8x less collective traffic + HBM read than AllGather.

Key layout choices per core:
  - qkv computed feature-major ([feat, tok]) via PE-transposed X;
  - all PE transposes run on float32r data (tf32 path, 1.5 cyc/row vs 2.0);
  - KIVI 2-bit fake-quant of K on device, batched 512 tokens at a time;
  - scores computed transposed ([kpos, q]); the two heads' 64-contraction
    score matmuls are interleaved so they land on PE row groups (0,*) and
    (64,*) and can stream concurrently;
  - softmax sum rides a ones-column in the AV matmul; normalization uses a
    fast-approx reciprocal + gpsimd partition_broadcast (no PE involvement).
"""
import sys
sys.path.insert(0, '/opt/trn_rl_repo')
import numpy as np

P = 128
TOK = 4096          # B*S = 4*1024
E = 1024
NB = 8              # embed 128-blocks
CH = 512            # token chunk
NCH = 8             # token 512-chunks
MAGIC = 8388608.0   # 2^23: x + MAGIC - MAGIC == rint(x) for 0 <= x < 2^22
DEBUG_TAPS = False

_CACHE = {}


def _build(sim_single=False):
    import concourse.bacc as bacc
    import concourse.mybir as mybir
    import concourse.tile as tile

    f32 = mybir.dt.float32
    fmm = mybir.dt.float32r
    bf16 = mybir.dt.bfloat16
    X = mybir.AxisListType.X
    ADD = mybir.AluOpType.add
    MULT = mybir.AluOpType.mult
    MAX = mybir.AluOpType.max
    SUB = mybir.AluOpType.subtract
    EXP = mybir.ActivationFunctionType.Exp

    nc = bacc.Bacc("TRN2", target_bir_lowering=False, debug=False,
                   num_devices=(1 if sim_single else 8))

    x_ap = nc.dram_tensor("x", [TOK, E], bf16, kind="ExternalInput").ap()
    wqkv_ap = nc.dram_tensor("wqkv", [E, 384], bf16, kind="ExternalInput").ap()
    bqkv_ap = nc.dram_tensor("bqkv", [P, 3], f32, kind="ExternalInput").ap()
    m8t_ap = nc.dram_tensor("m8t", [P, 32], f32, kind="ExternalInput").ap()
    wp_ap = nc.dram_tensor("wp", [E, E], bf16, kind="ExternalInput").ap()
    bp_ap = nc.dram_tensor("bp", [P, NB], f32, kind="ExternalInput").ap()
    ident_ap = nc.dram_tensor("ident", [P, P], bf16, kind="ExternalInput").ap()
    ones1_ap = nc.dram_tensor("ones1", [1, 64], fmm, kind="ExternalInput").ap()
    yt_ap = nc.dram_tensor("yt", [NB, P, CH], f32, kind="ExternalOutput").ap()
    if DEBUG_TAPS:
        dbg_q = nc.dram_tensor("dbg_q", [P, TOK], fmm, kind="ExternalOutput").ap()
        dbg_kd = nc.dram_tensor("dbg_kd", [P, TOK], fmm, kind="ExternalOutput").ap()
        dbg_o = nc.dram_tensor("dbg_o", [P, TOK], fmm, kind="ExternalOutput").ap()
        dbg_a = nc.dram_tensor("dbg_a", [NB, P, CH], fmm, kind="ExternalOutput").ap()

    with tile.TileContext(nc) as tc:
        with tc.tile_pool(name="const", bufs=1) as constp, \
             tc.tile_pool(name="big", bufs=1) as bigp, \
             tc.tile_pool(name="dram", bufs=1, space="DRAM") as dramp:

            identb = constp.tile([P, P], bf16)
            nc.sync.dma_start(identb[:], ident_ap)
            # chunk-0 x tiles loaded first so stage-1 transposes start ASAP;
            # the bulk constant/weight DMAs queue behind them
            xn0s = []
            for tb in range(4):
                xn = constp.tile([P, E], bf16, name=f"xn0_{tb}",
                                 tag=f"xn0_{tb}")
                nc.sync.dma_start(xn[:], x_ap[tb * P:(tb + 1) * P, :])
                xn0s.append(xn)
            wts = []
            for eb in range(NB):
                wt = constp.tile([P, 384], bf16, name=f"wt{eb}", tag=f"wt{eb}")
                nc.sync.dma_start(wt[:], wqkv_ap[eb * P:(eb + 1) * P, :])
                wts.append(wt)
            m8tt = constp.tile([P, 32], f32)
            nc.sync.dma_start(m8tt[:], m8t_ap)
            bqkvt = constp.tile([P, 3], f32)
            nc.sync.dma_start(bqkvt[:], bqkv_ap)
            bpt = constp.tile([P, NB], f32)
            nc.sync.dma_start(bpt[:], bp_ap)
            onescol = constp.tile([P, 1], f32)
            nc.any.memset(onescol[:], 1.0)
            ones1r = constp.tile([1, 64], fmm)
            nc.sync.dma_start(ones1r[:], ones1_ap)
            # wp tiles are declared here but loaded after stage 2 (below) so
            # the 4MB of weight DMA stays off the stage-1 critical path
            wps = []
            for fb in range(NB):
                wpt = constp.tile([P, E], bf16, name=f"wp{fb}", tag=f"wp{fb}")
                wps.append(wpt)

            # persistent feature-major tensors [128 = 2 heads x 64, 4096 tok]
            qT = bigp.tile([P, TOK], bf16, tag="qT")
            kdT = bigp.tile([P, TOK], bf16, tag="kdT")
            oT = bigp.tile([P, TOK], bf16, tag="oT")

            with tc.tile_pool(name="kv", bufs=1) as kvp:
                kT = kvp.tile([P, TOK], bf16, tag="kT")
                vT = kvp.tile([P, TOK], bf16, tag="vT")
                qkvT = [qT, kT, vT]

                # ------------- Stage 1: X^T + QKV^T GEMM ----------------
                with tc.tile_pool(name="s1", bufs=2) as s1p, \
                     tc.tile_pool(name="s1ps", bufs=2, space="PSUM") as s1ps, \
                     tc.tile_pool(name="g1ps", bufs=3, space="PSUM") as g1ps:
                    for ch in range(NCH):
                        if ch == 0:
                            xns = xn0s
                        else:
                            xns = []
                            for tb in range(4):
                                xn = s1p.tile([P, E], bf16, name=f"xn{tb}",
                                              tag=f"xn{tb}")
                                nc.sync.dma_start(
                                    xn[:],
                                    x_ap[ch * CH + tb * P:
                                         ch * CH + (tb + 1) * P, :])
                                xns.append(xn)
                        xTs = []
                        for eb in range(NB):
                            xT = s1p.tile([P, CH], bf16, name=f"xT{eb}",
                                          tag=f"xT{eb}")
                            xTs.append(xT)
                        for eb in range(NB):
                            ps_x = s1ps.tile([P, CH], bf16, tag="ps_x")
                            for tb in range(4):
                                nc.tensor.transpose(
                                    ps_x[:, tb * P:(tb + 1) * P],
                                    xns[tb][:, eb * P:(eb + 1) * P], identb[:])
                            if eb % 2 == 0:
                                nc.vector.tensor_copy(xTs[eb][:], ps_x[:])
                            else:
                                nc.scalar.copy(xTs[eb][:], ps_x[:])
                        for m in range(3):
                            gps = g1ps.tile([P, CH], f32, tag="gps")
                            for eb in range(NB):
                                nc.tensor.matmul(
                                    gps[:], wts[eb][:, m * P:(m + 1) * P],
                                    xTs[eb][:],
                                    start=(eb == 0), stop=(eb == NB - 1))
                            with nc.allow_low_precision(reason="tf32 store"):
                                nc.vector.tensor_tensor(
                                    qkvT[m][:, ch * CH:(ch + 1) * CH], gps[:],
                                    bqkvt[:, m:m + 1].to_broadcast((P, CH)),
                                    ADD)

                # V natural tiles, one per (kpos-block, head), ones col at 64
                vt_tiles = []
                for kb in range(TOK // P):
                    vh = []
                    for h in range(2):
                        v = bigp.tile([P, 65], bf16, name=f"v{kb}_{h}",
                                      tag=f"v{kb}_{h}")
                        nc.vector.tensor_copy(v[:, 64:65], onescol[:])
                        vh.append(v)
                    vt_tiles.append(vh)

                # ------- Stage 2: KIVI fake-quant of K, V transpose ------
                # batched 4 kpos-blocks (512 tokens) at a time
                with tc.tile_pool(name="s2", bufs=2) as s2p, \
                     tc.tile_pool(name="s2ps", bufs=2, space="PSUM") as s2ps, \
                     tc.tile_pool(name="s2ps2", bufs=2, space="PSUM") as s2ps2:
                    for g in range(NCH):
                        g0 = g * CH
                        ps_k = s2ps.tile([P, CH], bf16, tag="ps_k")
                        for j in range(4):
                            nc.tensor.transpose(
                                ps_k[:, j * P:(j + 1) * P],
                                kT[:, g0 + j * P:g0 + (j + 1) * P], identb[:])
                        knat = s2p.tile([P, CH], f32, tag="knat")
                        nc.scalar.copy(knat[:], ps_k[:])
                        gmax = s2p.tile([P, P], f32, tag="gmax")
                        nc.vector.tensor_reduce(
                            gmax[:], knat[:].rearrange("p (g f) -> p g f", f=4),
                            axis=X, op=MAX, apply_absolute_value=True)
                        scalet = s2p.tile([P, P], f32, tag="scalet")
                        nc.vector.tensor_scalar_mul(scalet[:], gmax[:],
                                                    1.0 / 1.5)
                        rs = s2p.tile([P, P], f32, tag="rs")
                        nc.vector.reciprocal_approx_fast(rs[:], scalet[:])
                        kd = s2p.tile([P, CH], f32, tag="kd")
                        kd_g = kd[:].rearrange("p (g f) -> p g f", f=4)
                        knat_g = knat[:].rearrange("p (g f) -> p g f", f=4)
                        nc.vector.tensor_tensor(
                            kd_g, knat_g, rs[:, :, None].to_broadcast((P, P, 4)),
                            MULT)
                        nc.vector.tensor_scalar(kd[:], kd[:], 1.5, MAGIC,
                                                ADD, ADD)
                        nc.vector.tensor_scalar(kd[:], kd[:], MAGIC, 1.5,
                                                SUB, SUB)
                        kdq = s2p.tile([P, CH], bf16, tag="kdq")
                        with nc.allow_low_precision(reason="bf16 store"):
                            nc.vector.tensor_tensor(
                                kdq[:].rearrange("p (g f) -> p g f", f=4), kd_g,
                                scalet[:, :, None].to_broadcast((P, P, 4)),
                                MULT)
                        ps_k2 = s2ps.tile([P, CH], bf16, tag="ps_k2")
                        for j in range(4):
                            nc.tensor.transpose(
                                ps_k2[:, j * P:(j + 1) * P],
                                kdq[:, j * P:(j + 1) * P], identb[:])
                        nc.scalar.copy(kdT[:, g0:g0 + CH], ps_k2[:])

                        ps_v = s2ps2.tile([P, CH], bf16, tag="ps_v")
                        for j in range(4):
                            nc.tensor.transpose(
                                ps_v[:, j * P:(j + 1) * P],
                                vT[:, g0 + j * P:g0 + (j + 1) * P], identb[:])
                        for j in range(4):
                            for h in range(2):
                                dst = vt_tiles[4 * g + j][h][:, 0:64]
                                src = ps_v[:, j * P + h * 64:j * P + (h + 1) * 64]
                                if (j + h) % 2 == 0:
                                    nc.scalar.copy(dst, src)
                                else:
                                    nc.vector.tensor_copy(dst, src)

            for fb in range(NB):
                nc.sync.dma_start(wps[fb][:], wp_ap[fb * P:(fb + 1) * P, :])

            # ---------------- Stage 4: attention ------------------------
            # per-batch AllToAll: core c's token slab interleaves across
            # batches ({b*1024 + c*128 .. +128}), so batch b's exchange fires
            # as soon as its attention output is done and overlaps the
            # attention compute of batches b+1..3
            a2a_ins = [dramp.tile([NB, P, P], bf16, name=f"a2a_in{b}",
                                  tag=f"a2a_in{b}") for b in range(4)]
            a2a_outs = [dramp.tile([NB, P, P], bf16, name=f"a2a_out{b}",
                                   tag=f"a2a_out{b}") for b in range(4)]
            recvs = []
            for fb in range(NB):
                rt = bigp.tile([P, CH], bf16, name=f"rt{fb}", tag=f"rt{fb}")
                recvs.append(rt)
            with tc.tile_pool(name="s4", bufs=2) as s4p, \
                 tc.tile_pool(name="s4ps", bufs=2, space="PSUM") as s4ps, \
                 tc.tile_pool(name="avps", bufs=3, space="PSUM") as avps, \
                 tc.tile_pool(name="rps", bufs=1, space="PSUM") as rps:
                for b in range(4):
                    for qc in range(2):
                        q0 = b * 1024 + qc * CH
                        es = []
                        for kb in range(8):
                            gkb = b * 8 + kb
                            # both heads' scores into one 2-bank PSUM tile;
                            # same kpos block => same mask bias, so a single
                            # EXP instruction covers both heads
                            ps_s = s4ps.tile([P, 2 * CH], f32, tag="ps_s")
                            for h in range(2):
                                hs = slice(h * 64, (h + 1) * 64)
                                nc.tensor.matmul(
                                    ps_s[:, h * CH:(h + 1) * CH],
                                    kdT[hs, gkb * P:(gkb + 1) * P],
                                    qT[hs, q0:q0 + CH],
                                    start=True, stop=True)
                            e = s4p.tile([P, 2 * CH], bf16,
                                         name=f"e{kb}", tag=f"e{kb}")
                            nc.scalar.activation(
                                e[:], ps_s[:], EXP,
                                bias=m8tt[:, gkb:gkb + 1], scale=0.125)
                            es.append(e)
                        for h in range(2):
                            hs = slice(h * 64, (h + 1) * 64)
                            ps_av = avps.tile([65, CH], f32, tag="ps_av")
                            for kb in range(8):
                                nc.tensor.matmul(
                                    ps_av[:], vt_tiles[b * 8 + kb][h][:],
                                    es[kb][:, h * CH:(h + 1) * CH],
                                    start=(kb == 0), stop=(kb == 7))
                            # replicate the RAW denominator row across 64
                            # partitions first (PE ones-matmul), then one
                            # full-width single-pass approx reciprocal — the
                            # single-lane [1,512] exact reciprocal took 3.3us
                            # on the critical path
                            denS = s4p.tile([1, CH], fmm, tag="denS")
                            with nc.allow_low_precision(reason="tf32 copy"):
                                nc.scalar.copy(denS[:], ps_av[64:65, :])
                            ps_r = rps.tile([64, CH], f32, tag="ps_r")
                            nc.tensor.matmul(ps_r[:], ones1r[:], denS[:],
                                             start=True, stop=True)
                            denrep = s4p.tile([64, CH], f32, tag="denrep")
                            nc.scalar.copy(denrep[:], ps_r[:])
                            rrep = s4p.tile([64, CH], f32, tag="rrep")
                            nc.vector.reciprocal_approx_fast(
                                rrep[:], denrep[:])
                            with nc.allow_low_precision(reason="tf32 store"):
                                nc.vector.tensor_tensor(
                                    oT[hs, q0:q0 + CH], ps_av[0:64, :],
                                    rrep[:], MULT)
                    # batch b's oT columns are final: exchange them now so the
                    # collective overlaps attention of the remaining batches
                    for j in range(NB):
                        nc.sync.dma_start(
                            a2a_ins[b][j],
                            oT[:, b * 1024 + j * P:b * 1024 + (j + 1) * P])
                    if sim_single:
                        for r in range(NB):
                            nc.gpsimd.dma_start(a2a_outs[b][r], a2a_ins[b][r])
                    else:
                        nc.gpsimd.collective_compute(
                            "AllToAll", mybir.AluOpType.bypass,
                            replica_groups=[list(range(8))],
                            ins=[a2a_ins[b][:]], outs=[a2a_outs[b][:]])
                    for src in range(NB):
                        nc.sync.dma_start(recvs[src][:, b * P:(b + 1) * P],
                                          a2a_outs[b][src])

            # ------------- Stage 5: token-sharded c_proj -----------------
            if DEBUG_TAPS:
                nc.sync.dma_start(dbg_q, qT[:])
                nc.sync.dma_start(dbg_kd, kdT[:])
                nc.sync.dma_start(dbg_o, oT[:])
            with tc.tile_pool(name="s5", bufs=1) as s5p, \
                 tc.tile_pool(name="s5ps", bufs=2, space="PSUM") as s5ps:
                rts = recvs
                # split the GEMM by token columns: batches 0-2 (cols 0:384)
                # depend only on the first three exchanges, so that 3/4 of
                # the projection runs during the batch-3 AllToAll window;
                # only the cols 384:512 pass sits after the last exchange
                W = 3 * P
                for eb in range(NB):
                    ps_p = s5ps.tile([P, W], f32, tag="ps_pA")
                    for fb in range(NB):
                        nc.tensor.matmul(ps_p[:],
                                         wps[fb][:, eb * P:(eb + 1) * P],
                                         rts[fb][:, 0:W],
                                         start=(fb == 0), stop=(fb == NB - 1))
                    yts = s5p.tile([P, W], f32, name=f"ytsA{eb}",
                                   tag=f"ytsA{eb % 2}")
                    nc.vector.tensor_tensor(
                        yts[:], ps_p[:],
                        bpt[:, eb:eb + 1].to_broadcast((P, W)), ADD)
                    nc.sync.dma_start(yt_ap[eb][:, 0:W], yts[:])
                for eb in range(NB):
                    ps_p = s5ps.tile([P, CH - W], f32, tag="ps_pB")
                    for fb in range(NB):
                        nc.tensor.matmul(ps_p[:],
                                         wps[fb][:, eb * P:(eb + 1) * P],
                                         rts[fb][:, W:CH],
                                         start=(fb == 0), stop=(fb == NB - 1))
                    yts = s5p.tile([P, CH - W], f32, name=f"ytsB{eb}",
                                   tag=f"ytsB{eb % 2}")
                    nc.vector.tensor_tensor(
                        yts[:], ps_p[:],
                        bpt[:, eb:eb + 1].to_broadcast((P, CH - W)), ADD)
                    nc.sync.dma_start(yt_ap[eb][:, W:CH], yts[:])

    nc.compile()
    return nc


def make_in_maps(hidden_states, attention_mask, w_attn, b_attn, w_proj, b_proj):
    import ml_dtypes
    bf = ml_dtypes.bfloat16
    x = np.ascontiguousarray(
        np.asarray(hidden_states, np.float32).reshape(TOK, E)).astype(bf)
    mask = np.asarray(attention_mask, np.float32)
    wa = np.asarray(w_attn, np.float32)
    ba = np.asarray(b_attn, np.float32)
    wpf = np.ascontiguousarray(np.asarray(w_proj, np.float32)).astype(bf)
    bp = np.asarray(b_proj, np.float32)

    m8 = (mask * np.float32(0.125)).reshape(4, 8, 128)
    m8t = np.ascontiguousarray(m8.transpose(2, 0, 1).reshape(128, 32))
    ident = np.eye(P, dtype=bf)
    ones1 = np.ones((1, 64), dtype=np.float32)
    bp_pack = np.ascontiguousarray(bp.reshape(NB, P).T)

    in_maps = []
    for c in range(8):
        cs = slice(c * P, (c + 1) * P)
        wqkv = np.ascontiguousarray(np.concatenate(
            [wa[:, cs], wa[:, 1024 + c * P:1024 + (c + 1) * P],
             wa[:, 2048 + c * P:2048 + (c + 1) * P]], axis=1)).astype(bf)
        bqkv = np.ascontiguousarray(np.stack(
            [ba[cs], ba[1024 + c * P:1024 + (c + 1) * P],
             ba[2048 + c * P:2048 + (c + 1) * P]], axis=1))
        in_maps.append({
            "x": x, "wqkv": wqkv, "bqkv": bqkv, "m8t": m8t,
            "wp": wpf, "bp": bp_pack, "ident": ident, "ones1": ones1,
        })
    return in_maps


def kernel(hidden_states, attention_mask, w_attn, b_attn, w_proj, b_proj):
    from concourse import bass_utils
    if "nc" not in _CACHE:
        _CACHE["nc"] = _build()
    nc = _CACHE["nc"]
    in_maps = make_in_maps(hidden_states, attention_mask, w_attn, b_attn,
                           w_proj, b_proj)
    res = bass_utils.run_bass_kernel_spmd(nc, in_maps, core_ids=list(range(8)))
    y = np.empty((TOK, E), dtype=np.float32)
    for c in range(8):
        blk = res.results[c]["yt"].reshape(E, 4, P)  # [feat, batch, tok]
        for b in range(4):
            y[b * 1024 + c * P:b * 1024 + (c + 1) * P, :] = blk[:, b, :].T
    return y.reshape(4, 1024, E)
